# revision 1
# baseline (speedup 1.0000x reference)
"""Trainium2 Bass kernel for DigitConvolutionalModel.

Model: x[B,784] -> reshape 28x28 -> 3x3 valid conv (weights conv_w) ->
[B,676] -> Linear(676,100)+relu -> Linear(100,10)+relu -> Linear(10,10).

The conv is linear, so it folds into the first Linear: W1f = C @ w1 where
C[784,676] is the conv unfold matrix. The whole model becomes a 3-layer MLP
784 -> 100 -> 10 -> 10 with relu between layers.

Sharding: pure data parallel, batch split across 8 cores (8192 rows each).

Precision: x is cast host-side to fp8 e3m4 (4 mantissa bits) — halves HBM
traffic vs bf16; weights stay bf16 (mixed-dtype matmul streams at the same
1 cycle/row). Measured end-to-end rel err 0.0142 vs the 2e-2 gate.

PE work per 512-batch supertile t is SEVEN 512-row matmuls:
  - 6 L1 main chunks (128 features each, features 0..767), accumulating
    into PSUM bank(t) rows 0-99 with start=False.
  - 1 "fused" matmul with block stationary S[126,126]:
        rows   0..99  = W2   -> out cols 100..109   (L2 of supertile t)
        rows 100..109 = W3   -> out cols 116..125   (L3 of supertile t-4)
        rows 110..125 = W1t  -> out cols   0..99    (L1 tail of t+4)
    moving operand fmov(t)[126, 512] (bf16, slot t of one persistent tile):
        rows   0..99  = h1(t)      (ACT relu of bank(t) rows 0..99)
        rows 100..109 = h2(t-4)    (ACT relu of an earlier fused output)
        rows 110..125 = xtail(t+4) (features 768..783, one slot-arranged
                                    DMA at kernel start)
    out = PSUM bank(t+4): rows 0..99 initialize the L1 accumulation for
    supertile t+4 (start=True; its chunks follow with start=False), rows
    100..109 -> relu -> h2(t), rows 116..125 + b3 -> y(t-4) -> store.
  Banks 0..3 are seeded by 4 standalone W1t tail matmuls at the start;
  supertiles 12..15 drain through 6 extra fused passes (scratch banks).

Biases live in the weight blob at the partition rows where the ACT engine
reads them: b1 rows 0-99, b2 rows 100-109, b3 rows 116-125. Engine APs
need a base partition in {0,32,64,96}, so the small h2/y ACTs run from
base 96: h2 as a 14-row op [96:110) whose garbage lanes 96..99 are
overwritten by the (later) h1 ACT of the same fmov slot — crucially it
stops BEFORE the xtail rows at 110+ — and y as a 30-row op [96:126)
into a private tile where only rows 116..125 are stored.
"""

import numpy as np
import ml_dtypes

import concourse.bacc as bacc
import concourse.tile as tile
from concourse.tile import add_dep_helper
from concourse import mybir
from concourse.bass_utils import run_bass_kernel_spmd

N_CORES = 8
B = 65536
BC = B // N_CORES  # 8192 rows per core
TN = 512           # batch columns per supertile
NT = BC // TN      # 16 supertiles per core
NKC = 6            # full 128-feature chunks (0..767)
KT = 16            # tail features (768..783)
NF = 784
H1 = 100
HO = 10
F32 = mybir.dt.float32
BF16 = mybir.dt.bfloat16
F8E3 = mybir.dt.float8e3
NP_BF16 = ml_dtypes.bfloat16
NP_F8E3 = ml_dtypes.float8_e3m4

# packed weight blob column layout (bf16 columns)
_C_W1M = 0                      # [128, 600]  w1m chunks
_C_FS = 600                     # [126, 126]  fused stationary S
_C_W1T = 726                    # [16, 100]   w1t (standalone, banks 0-3)
_C_B = 826                      # [126, 2]    b1/b3/b2 f32 byte-pairs by row
WBW = 828

NPAIR = NT // 2


def _build_nc():
    nc = bacc.Bacc(None, target_bir_lowering=False)

    xt_main = nc.dram_tensor(
        "xt_main", [NT, 128, NKC, TN], F8E3, kind="ExternalInput"
    )
    # tails arranged by fmov slot s (holding xtail(s+4); zeros for s>=12),
    # bf16 so they ride the bf16 fmov tile
    xt_tail = nc.dram_tensor("xt_tail", [KT, 22, TN], BF16, kind="ExternalInput")
    # tails 0..3 for the standalone bank-seed matmuls
    xt_tl03 = nc.dram_tensor("xt_tl03", [KT, 4, TN], BF16, kind="ExternalInput")
    wblob = nc.dram_tensor("wblob", [128, WBW], BF16, kind="ExternalInput")
    yt = nc.dram_tensor("yt", [HO, BC], F32, kind="ExternalOutput")

    relu = mybir.ActivationFunctionType.Relu
    ident = mybir.ActivationFunctionType.Identity

    with tile.TileContext(nc) as tc:
        with (
            tc.tile_pool(name="const", bufs=1) as cpool,
            tc.tile_pool(name="xm", bufs=6) as xpool,
            tc.tile_pool(name="fm", bufs=6) as fpool,
            tc.tile_pool(name="ot", bufs=4) as opool,
            tc.tile_pool(name="psA", bufs=6, space="PSUM") as psA,
        ):
            # weights on the sync queue: the small hot block (fused
            # stationary, w1t, biases — 58KB) goes first so the first
            # cold-queue DMA completion gates only a short transfer;
            # the 154KB of L1 chunk weights follow.
            wb_s = cpool.tile([128, WBW], BF16, tag="wb")
            nc.sync.dma_start(wb_s[:, _C_FS:], wblob[:, _C_FS:])
            nc.sync.dma_start(wb_s[:, 0:_C_FS], wblob[:, 0:_C_FS])
            xtl03 = cpool.tile([KT, 4, TN], BF16, tag="xtl03")
            nc.gpsimd.dma_start(xtl03[:], xt_tl03[:])

            # fused-moving tiles are pooled PER PAIR (dep tracking
            # intersects partition ranges coarsely — one persistent tile
            # would make every fused matmul wait on the latest h1/h2
            # write to ANY slot, serializing PE behind ACT each pair).
            # ptile q covers fmov slots 2q / 2q+1; its writers (h1-ACT of
            # pair q, h2-DVE from pair q-2, tail DMA) all complete at
            # least a pair before fused reads it.
            ptiles: dict[int, object] = {}

            def alloc_ptile(q):
                pt = fpool.tile([126, 2, TN], BF16, tag="fm",
                                name=f"ptile{q}")
                ptiles[q] = pt
                # tails for fmov slots 2q/2q+1 (zeros for slots >= 12)
                nc.gpsimd.dma_start(pt[110:126, :, :],
                                    xt_tail[:, 2 * q:2 * q + 2, :])
                if q <= 2:
                    # no h2(-6..-1) exist for fused(0..5)
                    nc.vector.memset(pt[96:110, :, :], 0.0)
                if q >= 8:
                    # drain slots 16..19: h1 region is never produced
                    nc.vector.memset(pt[0:H1, :, :], 0.0)
                return pt

            def fmov(s):
                return ptiles[s // 2][:, s % 2, :]

            fs_ap = wb_s[0:126, _C_FS:_C_FS + 126]
            w1t_ap = wb_s[0:KT, _C_W1T:_C_W1T + H1]
            b1_ap = wb_s[0:H1, _C_B:_C_B + 2].bitcast(F32)
            # engine APs need a base partition in {0,32,64,96}: the small
            # h2/y ACTs run from base 96 (lanes are parallel, width free);
            # rows 96..99 are garbage lanes, overwritten (fmov h1) or never
            # stored (ot). One bias column serves all: b1 rows 0-99,
            # b2 rows 100-109, b3 rows 116-125.
            bq2_ap = wb_s[96:110, _C_B:_C_B + 2].bitcast(F32)
            bq_ap = wb_s[96:126, _C_B:_C_B + 2].bitcast(F32)

            prev_mm = [None]

            def mm(out_ap, lhsT_ap, rhs_ap, start, stop, ldw=True):
                m = nc.tensor.matmul(out_ap, lhsT_ap, rhs_ap,
                                     start=start, stop=stop,
                                     skip_group_check=True)
                if not ldw:
                    m.ins.ldweights = False
                if prev_mm[0] is not None:
                    add_dep_helper(m.ins, prev_mm[0], sync=False,
                                   reason="pe program order")
                prev_mm[0] = m.ins
                return m

            # Short warmup: covers engine bring-up until the first real
            # operands land (~1.5us); PE pstate then ramps on real work.
            wsc = cpool.tile([128, TN], BF16, tag="wsc")
            wp0 = psA.tile([126, TN], F32, tag="pa")
            wp1 = psA.tile([126, TN], F32, tag="pa")
            wfirst = nc.tensor.matmul(wp0[:], wsc[:, 0:126], wsc[:],
                                      start=True, stop=True)
            for i in range(1, 4):
                w_mm = nc.tensor.matmul((wp1 if i % 2 else wp0)[:],
                                        wsc[:, 0:126], wsc[:],
                                        start=True, stop=True)
                w_mm.ins.ldweights = False
                add_dep_helper(w_mm.ins, wfirst.ins, sync=False,
                               reason="warmup weight reuse")
            # WAR on purpose: warmup multiplies garbage; the memset only
            # exists to satisfy tile allocation and runs afterwards.
            nc.vector.memset(wsc[:], 0.0)

            banks: dict[int, object] = {}
            ots: dict[int, object] = {}

            alloc_ptile(0)
            alloc_ptile(1)

            # Seed banks 0..3 with their L1 tail contribution.
            for t in range(4):
                banks[t] = psA.tile([126, TN], F32, tag="pa", name=f"bank{t}")
                mm(banks[t][0:H1, :], w1t_ap, xtl03[:, t, :],
                   start=True, stop=False, ldw=(t == 0))

            def emit_fused(s, ldw):
                """fused(s): L2(s) + L3(s-6) + L1-tail(s+4) -> bank(s+4)."""
                bk = psA.tile([126, TN], F32, tag="pa", name=f"bank{s+4}")
                banks[s + 4] = bk
                mm(bk[:], fs_ap, fmov(s),
                   start=True, stop=(s + 4 > 15), ldw=ldw)

            def emit_post(s):
                """h2/y reads of bank(s+4). Emitted AFTER the pair's
                chunks: their base-96 APs touch rows 96..99 of the bank,
                and emitting them first would WAR-block the chunk matmuls
                behind two ~700ns engine ops every pair."""
                bk = banks[s + 4]
                if s <= 15:
                    # h2(s) -> fmov(s+6) rows 100..109 (14-row base-96 op;
                    # rows 96..99 garbage, overwritten by h1-ACT(s+6);
                    # stops BEFORE the xtail rows at 110+). On DVE: ops
                    # cost ~free-size regardless of rows, and the scalar
                    # queue is busy with the h1 relus + y adds. The read
                    # of rows 96..99 RAW-depends on bank(s+4)'s own
                    # chunks (same pair) — hence the SIX-supertile h2 lag,
                    # which leaves ~3 pairs of slack before fused(s+6).
                    nc.vector.scalar_tensor_tensor(
                        ptiles[(s + 6) // 2][96:110, s % 2, :],
                        bk[96:110, :], bq2_ap, wsc[96:110, :],
                        op0=mybir.AluOpType.add, op1=mybir.AluOpType.max)
                if 6 <= s < 12:
                    ot = opool.tile([126, TN], F32, tag="ot", name=f"ot{s-6}")
                    nc.scalar.activation(ot[96:126, :], bk[96:126, :],
                                         ident, bias=bq_ap)
                    # stores must not sit in the sync (loads) in-order
                    # queue: a store waiting on the L3 chain would gate
                    # later x-tile loads. gpsimd is idle.
                    nc.gpsimd.dma_start(
                        yt[:, (s - 6) * TN:(s - 5) * TN], ot[116:126, :]
                    )
                    ots[s - 6] = ot
                elif s >= 12:
                    # drain: y(6..10) batch on ACT, y(11..15) on DVE —
                    # two wide tiles, ONE store each (a per-y store
                    # would put ten ~800ns DMA issues after the last
                    # matmul). Both stores ride the by-then-idle sync
                    # queue.
                    if s <= 16:
                        nc.scalar.activation(
                            ot_act[96:126, s - 12, :], bk[96:126, :],
                            ident, bias=bq_ap)
                        if s == 16:
                            nc.sync.dma_start(
                                yt[:, 6 * TN:11 * TN],
                                ot_act[116:126, :, :])
                    else:
                        nc.vector.scalar_tensor_tensor(
                            ot_dve[96:126, s - 17, :], bk[96:126, :],
                            bq_ap, wsc[96:126, :],
                            op0=mybir.AluOpType.add,
                            op1=mybir.AluOpType.add)
                        if s == 21:
                            nc.sync.dma_start(
                                yt[:, 11 * TN:16 * TN],
                                ot_dve[116:126, :, :])

            for p in range(NPAIR):
                t0, t1 = 2 * p, 2 * p + 1
                alloc_ptile(p + 2)
                fm = ptiles[p]
                # fused passes for the pair-before-last: every dependency
                # (h1-ACT of pair p-2, h2 writes from pair p-1) is at
                # least a full pair old, so the PE never waits here, and
                # the y-ACTs land in the scalar queue's idle early-pair
                # window, ahead of the late-pair h1 relus.
                if p >= 2:
                    emit_fused(2 * p - 4, ldw=True)
                    emit_fused(2 * p - 3, ldw=False)
                xm0 = xpool.tile([128, NKC, TN], F8E3, tag="xm")
                xm1 = xpool.tile([128, NKC, TN], F8E3, tag="xm")
                if p == 0:
                    # split first supertile so chunk 0 can start earlier
                    nc.sync.dma_start(xm0[:, 0:3, :], xt_main[t0, :, 0:3, :])
                    nc.sync.dma_start(xm0[:, 3:6, :], xt_main[t0, :, 3:6, :])
                else:
                    nc.sync.dma_start(xm0[:], xt_main[t0])
                nc.sync.dma_start(xm1[:], xt_main[t1])

                if p == NPAIR - 1:
                    # last pair: all of supertile 15 first so its h1/h2
                    # chain completes during supertile 14's chunks,
                    # shortening the drain.
                    for k in range(NKC):
                        mm(banks[t1][0:H1, :],
                           wb_s[:, k * H1:(k + 1) * H1],
                           xm1[:, k, :], start=False, stop=(k == NKC - 1))
                    nc.scalar.activation(fm[0:H1, 1, :],
                                         banks[t1][0:H1, :],
                                         relu, bias=b1_ap)
                    for k in range(NKC):
                        mm(banks[t0][0:H1, :],
                           wb_s[:, k * H1:(k + 1) * H1],
                           xm0[:, k, :], start=False, stop=(k == NKC - 1))
                    nc.scalar.activation(fm[0:H1, 0, :],
                                         banks[t0][0:H1, :],
                                         relu, bias=b1_ap)
                else:
                    for k in range(NKC):
                        mm(banks[t0][0:H1, :],
                           wb_s[:, k * H1:(k + 1) * H1],
                           xm0[:, k, :], start=False, stop=(k == NKC - 1))
                        mm(banks[t1][0:H1, :],
                           wb_s[:, k * H1:(k + 1) * H1],
                           xm1[:, k, :], start=False, stop=(k == NKC - 1),
                           ldw=False)
                    nc.scalar.activation(fm[0:H1, 0, :],
                                         banks[t0][0:H1, :],
                                         relu, bias=b1_ap)
                    nc.scalar.activation(fm[0:H1, 1, :],
                                         banks[t1][0:H1, :],
                                         relu, bias=b1_ap)
                # deferred bank readers for this pair's fused outputs
                if p >= 2:
                    emit_post(2 * p - 4)
                    emit_post(2 * p - 3)
                del banks[t0], banks[t1]

            # drain: fused(12..21) produce y(6..15); scratch banks have
            # no chunks, so each post can follow its matmul directly
            alloc_ptile(10)
            ot_act = opool.tile([126, 5, TN], F32, tag="ot", name="ot_act")
            ot_dve = opool.tile([126, 5, TN], F32, tag="ot", name="ot_dve")
            for s in range(12, 22):
                emit_fused(s, ldw=(s == 12))
                emit_post(s)

    nc.compile()
    return nc


def _fold_conv_into_w1(conv_w: np.ndarray, w1: np.ndarray) -> np.ndarray:
    """W1f[784,100] such that x @ W1f == conv(x).reshape(B,676) @ w1."""
    c = np.zeros((NF, 26 * 26), dtype=np.float64)
    for di in range(3):
        for dj in range(3):
            ii, jj = np.meshgrid(np.arange(26), np.arange(26), indexing="ij")
            src = (ii + di) * 28 + (jj + dj)
            dst = ii * 26 + jj
            c[src.ravel(), dst.ravel()] += np.float64(conv_w[di, dj])
    return (c @ w1.astype(np.float64)).astype(np.float32)


def _prep_in_maps(x, conv_w, w1, b1, w2, b2, w3, b3):
    x = np.asarray(x, dtype=np.float32)
    conv_w = np.asarray(conv_w, dtype=np.float32)
    w1 = np.asarray(w1, dtype=np.float32)
    b1 = np.asarray(b1, dtype=np.float32)
    w2 = np.asarray(w2, dtype=np.float32)
    b2 = np.asarray(b2, dtype=np.float32)
    w3 = np.asarray(w3, dtype=np.float32)
    b3 = np.asarray(b3, dtype=np.float32)

    w1f = _fold_conv_into_w1(conv_w, w1)  # [784, 100]
    # main chunks: feature f = k*128 + p -> [128, 600]
    w1m = np.ascontiguousarray(
        w1f[: 128 * NKC].reshape(NKC, 128, H1).transpose(1, 0, 2)
    ).astype(NP_BF16).reshape(128, NKC * H1)
    w1t = w1f[128 * NKC:].astype(NP_BF16)  # [16, 100]

    blob = np.zeros((128, WBW), np.uint16)
    blob[:, _C_W1M:_C_W1M + NKC * H1] = w1m.view(np.uint16)
    # fused stationary S[126,126]
    s_blk = np.zeros((126, 126), np.float32)
    s_blk[0:H1, 100:110] = w2
    s_blk[100:110, 116:126] = w3
    s_blk[110:126, 0:H1] = w1t.astype(np.float32)
    blob[0:126, _C_FS:_C_FS + 126] = s_blk.astype(NP_BF16).view(np.uint16)
    blob[0:KT, _C_W1T:_C_W1T + H1] = w1t.view(np.uint16)
    bias_rows = np.zeros((126, 1), np.float32)
    bias_rows[0:H1, 0] = b1
    bias_rows[100:110, 0] = b2
    bias_rows[116:126, 0] = b3
    blob[0:126, _C_B:_C_B + 2] = bias_rows.view(np.uint16)
    shared = {"wblob": blob.view(NP_BF16)}

    xb = x.astype(NP_F8E3)  # cast once, full batch
    in_maps = []
    for core in range(N_CORES):
        xc = xb[core * BC:(core + 1) * BC]  # [8192, 784] f8e3
        xct = xc.reshape(NT, TN, NF).transpose(0, 2, 1)  # [NT, NF, TN]
        xt_main = np.ascontiguousarray(
            xct[:, : 128 * NKC].reshape(NT, NKC, 128, TN).transpose(0, 2, 1, 3)
        )  # [NT, 128, NKC, TN]
        tails = xct[:, 128 * NKC:].astype(NP_BF16)  # [NT, KT, TN]
        # fmov slot s holds xtail(s+4); slots 12..21 stay zero
        xt_tail = np.zeros((KT, 22, TN), NP_BF16)
        xt_tail[:, 0:12, :] = tails[4:16].transpose(1, 0, 2)
        xt_tl03 = np.ascontiguousarray(tails[0:4].transpose(1, 0, 2))
        in_maps.append({"xt_main": xt_main, "xt_tail": xt_tail,
                        "xt_tl03": xt_tl03, **shared})
    return in_maps


_NC = None


def _get_nc():
    global _NC
    if _NC is None:
        _NC = _build_nc()
    return _NC


def kernel(x, conv_w, w1, b1, w2, b2, w3, b3):
    in_maps = _prep_in_maps(x, conv_w, w1, b1, w2, b2, w3, b3)
    nc = _get_nc()
    res = run_bass_kernel_spmd(nc, in_maps, core_ids=list(range(N_CORES)))
    out = np.empty((B, HO), dtype=np.float32)
    for i in range(N_CORES):
        out[i * BC:(i + 1) * BC] = res.results[i]["yt"].T
    return out


if __name__ == "__main__":
    rng = np.random.default_rng(0)
    inputs = {
        "x": rng.standard_normal((B, NF), dtype=np.float32),
        "conv_w": np.ones((3, 3), dtype=np.float32),
        "w1": (rng.standard_normal((676, H1)) * 0.04).astype(np.float32),
        "b1": np.zeros(H1, dtype=np.float32),
        "w2": (rng.standard_normal((H1, HO)) * 0.1).astype(np.float32),
        "b2": np.zeros(HO, dtype=np.float32),
        "w3": (rng.standard_normal((HO, HO)) * 0.3).astype(np.float32),
        "b3": np.zeros(HO, dtype=np.float32),
    }
    out = kernel(**inputs)
    print(out.shape, out.dtype)



# revision 5
# speedup vs baseline: 1.2707x; 1.2707x over previous
"""Trainium2 Bass kernel for DigitConvolutionalModel.

Model: x[B,784] -> reshape 28x28 -> 3x3 valid conv (weights conv_w) ->
[B,676] -> Linear(676,100)+relu -> Linear(100,10)+relu -> Linear(10,10).

The conv is linear, so it folds into the first Linear: W1f = C @ w1 where
C[784,676] is the conv unfold matrix. The whole model becomes a 3-layer MLP
784 -> 100 -> 10 -> 10 with relu between layers.

Sharding: pure data parallel, batch split across 8 cores (8192 rows each).

Precision: x is cast host-side to fp8 e3m4 — halves HBM traffic vs bf16;
weights stay bf16 (mixed-dtype matmul streams at 1 cycle/row). Measured
end-to-end rel err 0.0142 vs the 2e-2 gate.

PE work per 512-batch supertile t:
  - 6 L1 chunk matmuls (128 features each, 0..767) accumulating into
    PSUM bank(t) rows 0-99.
  - 1 fused matmul with block stationary S[126,126] (w2 rows 0:100 ->
    cols 100:110, w3 rows 100:110 -> cols 116:126, w1-tail rows
    110:126 -> cols 0:100), moving fmov(s)[126,512] (h1(s) rows 0:100,
    h2(s-6) rows 100:110, xtail(s+4) rows 110:126) -> bank(s+4):
    initializes the L1 accumulation of supertile s+4 (start=True),
    rows 100:110 -> relu -> h2(s), rows 116:126 + b3 -> y(s-6).
  Supertiles 0..3 are seeded by standalone w1-tail matmuls; y(0..3)
  come from the fused passes' L3 block (per-supertile ACT + store in
  ACT's idle mid-loop windows).

The remaining outputs y(4,5,14,15) and y(6..13) are PARTITION-STACKED:
h2(12..15) come from four L2-only fused passes (stationary S again;
the w1t-input rows of those fmov slots are zero) into scratch banks +
DVE relu. Then twelve L3 passes with a shifted-slice stationary SYW
(w3 at out-columns 10m..10m+10, zeros elsewhere — one [110,190]
zero-padded matrix sliced at different column offsets serves all
passes) accumulate y(6..13) into ONE psum bank at rows 10j..10j+10
and y(15,4,5,14) into a second bank. One wide ACT (+b3 replicated per
10-row group) and one store per bank replace twelve narrow
per-supertile ops — ACT/DVE cost is per-free-dim, so stacking
supertiles across partitions makes 12 outputs cost ~2 ops.

Biases live in the weight blob at the partition rows where the engines
read them; engine APs need a base partition in {0,32,64,96}, so the
small h2 relu ops run from base 96 with garbage lanes 96:100 that are
either overwritten later or feed zero-weight stationary rows.

Opening: warmup matmuls on garbage keep the PE busy from the first
post-preamble slot (~7.3us) until real operands land (~9.5us), so the
HAM activity monitor unthrottles the PE clock to 2.4GHz early; the
weight/x DMAs are interleaved so supertile-0 chunk 0 arrives ASAP.
"""

import numpy as np
import ml_dtypes

import concourse.bacc as bacc
import concourse.tile as tile
from concourse.tile import add_dep_helper
from concourse import mybir
from concourse.bass_utils import run_bass_kernel_spmd

N_CORES = 8
B = 65536
BC = B // N_CORES  # 8192 rows per core
TN = 512           # batch columns per supertile
NT = BC // TN      # 16 supertiles per core
NKC = 6            # full 128-feature chunks (0..767)
KT = 16            # tail features (768..783)
NF = 784
H1 = 100
HO = 10
F32 = mybir.dt.float32
BF16 = mybir.dt.bfloat16
F8E3 = mybir.dt.float8e3
NP_BF16 = ml_dtypes.bfloat16
NP_F8E3 = ml_dtypes.float8_e3m4

# packed weight blob column layout (bf16 columns)
_C_W1M = 0                      # [128, 600]  w1m chunks
_C_FS = 600                     # [126, 126]  fused stationary S
_C_W1T = 726                    # [16, 100]   w1t (standalone, banks 0-3)
_C_B = 826                      # [126, 2]    b1/b3/b2 f32 byte-pairs by row
_C_SY = 828                     # [110, 190]  SYW: w3 at rows 100:110, cols 90:100
_C_B3R = 1018                   # [100, 2]    b3 replicated per 10-row group, f32
WBW = 1020

NPAIR = NT // 2


def _build_nc():
    nc = bacc.Bacc(None, target_bir_lowering=False)

    xt_main = nc.dram_tensor(
        "xt_main", [NT, 128, NKC, TN], F8E3, kind="ExternalInput"
    )
    # tails arranged by fmov slot s (holding xtail(s+4); zeros for s>=12),
    # bf16 so they ride the bf16 fmov tile. Slots 0..15 only: slots 16+
    # are never streamed below row 110.
    xt_tail = nc.dram_tensor("xt_tail", [KT, 16, TN], BF16, kind="ExternalInput")
    # tails 0..3 for the standalone bank-seed matmuls
    xt_tl03 = nc.dram_tensor("xt_tl03", [KT, 4, TN], BF16, kind="ExternalInput")
    wblob = nc.dram_tensor("wblob", [128, WBW], BF16, kind="ExternalInput")
    yt = nc.dram_tensor("yt", [HO, 4 * TN], F32, kind="ExternalOutput")
    yt2 = nc.dram_tensor("yt2", [80, TN], F32, kind="ExternalOutput")
    yt3 = nc.dram_tensor("yt3", [40, TN], F32, kind="ExternalOutput")

    relu = mybir.ActivationFunctionType.Relu
    ident = mybir.ActivationFunctionType.Identity

    with tile.TileContext(nc) as tc:
        with (
            tc.tile_pool(name="const", bufs=1) as cpool,
            tc.tile_pool(name="xm", bufs=6) as xpool,
            tc.tile_pool(name="fm", bufs=6) as fpool,
            tc.tile_pool(name="ot", bufs=4) as opool,
            tc.tile_pool(name="psA", bufs=6, space="PSUM") as psA,
            tc.tile_pool(name="psY", bufs=2, space="PSUM") as psY,
        ):
            # Weight DMAs on the sync queue, interleaved with the first x
            # tiles so the first real matmuls can start ~9.5us: the small
            # hot block (stationary S, w1t, biases, SYW) first, then the
            # first two supertile-0 chunks, then the big w1m block.
            wb_s = cpool.tile([128, WBW], BF16, tag="wb")
            nc.sync.dma_start(wb_s[:, _C_FS:], wblob[:, _C_FS:])
            xtl03 = cpool.tile([KT, 4, TN], BF16, tag="xtl03")
            nc.gpsimd.dma_start(xtl03[:], xt_tl03[:])

            xm0 = xpool.tile([128, NKC, TN], F8E3, tag="xm")
            nc.sync.dma_start(xm0[:, 0:2, :], xt_main[0, :, 0:2, :])
            nc.sync.dma_start(wb_s[:, 0:_C_FS], wblob[:, 0:_C_FS])
            nc.sync.dma_start(xm0[:, 2:6, :], xt_main[0, :, 2:6, :])

            # fused-moving tiles pooled PER PAIR (dep tracking intersects
            # partition ranges coarsely — one persistent tile would make
            # every fused matmul wait on the latest h1/h2 write to ANY
            # slot). ptile q covers fmov slots 2q / 2q+1.
            ptiles: dict[int, object] = {}

            def alloc_ptile(q):
                pt = fpool.tile([126, 2, TN], BF16, tag="fm",
                                name=f"ptile{q}")
                ptiles[q] = pt
                if q <= 7:
                    # tails for fmov slots 2q/2q+1 (zeros for slots >= 12)
                    nc.gpsimd.dma_start(pt[110:126, :, :],
                                        xt_tail[:, 2 * q:2 * q + 2, :])
                else:
                    # slots 16..21 are only ever streamed as [0:110):
                    # memset the h1 region (h2 rows 96:110 written by STT)
                    nc.vector.memset(pt[0:96, :, :], 0.0)
                if q <= 2:
                    # no h2(-6..-1) exist for fused(0..5)
                    nc.vector.memset(pt[96:110, :, :], 0.0)
                return pt

            def fmov(s):
                return ptiles[s // 2][:, s % 2, :]

            fs_ap = wb_s[0:126, _C_FS:_C_FS + 126]
            w1t_ap = wb_s[0:KT, _C_W1T:_C_W1T + H1]
            b1_ap = wb_s[0:H1, _C_B:_C_B + 2].bitcast(F32)
            # engine APs need a base partition in {0,32,64,96}: the small
            # h2/y ops run from base 96 (lanes parallel, width free);
            # rows 96:100 are garbage lanes, overwritten (fmov h1) or
            # never stored. One bias column serves b1/b2/b3 by row.
            bq2_ap = wb_s[96:110, _C_B:_C_B + 2].bitcast(F32)
            bq_ap = wb_s[96:126, _C_B:_C_B + 2].bitcast(F32)
            b3rA_ap = wb_s[0:80, _C_B3R:_C_B3R + 2].bitcast(F32)
            b3rB_ap = wb_s[0:40, _C_B3R:_C_B3R + 2].bitcast(F32)

            def syA_ap(j):
                # stationary for stacked L3 pass j=0..7 (out [0:100)):
                # w3 lands at out columns 10j..10j+10, zeros elsewhere
                c = _C_SY + 90 - 10 * j
                return wb_s[0:110, c:c + 100]

            def syB_ap(m):
                # group B (out [0:40)): w3 at out columns 10m..10m+10
                c = _C_SY + 90 - 10 * m
                return wb_s[0:110, c:c + 40]

            prev_mm = [None]

            def mm(out_ap, lhsT_ap, rhs_ap, start, stop):
                m = nc.tensor.matmul(out_ap, lhsT_ap, rhs_ap,
                                     start=start, stop=stop,
                                     skip_group_check=True)
                if prev_mm[0] is not None:
                    add_dep_helper(m.ins, prev_mm[0], sync=False,
                                   reason="pe program order")
                prev_mm[0] = m.ins
                return m

            # Warmup on garbage (WAR on purpose: the memset below only
            # exists to satisfy tile allocation and runs afterwards).
            wsc = cpool.tile([128, TN], BF16, tag="wsc")
            wp0 = psA.tile([126, TN], F32, tag="pa")
            wp1 = psA.tile([126, TN], F32, tag="pa")
            wfirst = nc.tensor.matmul(wp0[:], wsc[:, 0:126], wsc[:],
                                      start=True, stop=True)
            prev_mm[0] = wfirst.ins
            for i in range(1, 5):
                w_mm = nc.tensor.matmul((wp1 if i % 2 else wp0)[:],
                                        wsc[:, 0:126], wsc[:],
                                        start=True, stop=True)
                add_dep_helper(w_mm.ins, wfirst.ins, sync=False,
                               reason="warmup weight reuse")
                prev_mm[0] = w_mm.ins
            nc.vector.memset(wsc[:], 0.0)

            banks: dict[int, object] = {}
            ots: dict[int, object] = {}

            alloc_ptile(0)
            alloc_ptile(1)

            # Seed banks 0..3 with their L1 tail contribution.
            for t in range(4):
                banks[t] = psA.tile([126, TN], F32, tag="pa", name=f"bank{t}")
                mm(banks[t][0:H1, :], w1t_ap, xtl03[:, t, :],
                   start=True, stop=False)

            def emit_fused(s):
                """fused(s): L2(s) + L3(s-6) + L1-tail(s+4) -> bank(s+4).
                For s>=12 the L1-tail input rows are zero and the bank is
                drain scratch (L2 rows 100:110 + garbage elsewhere)."""
                bk = psA.tile([126, TN], F32, tag="pa", name=f"bank{s+4}")
                banks[s + 4] = bk
                mm(bk[:], fs_ap, fmov(s),
                   start=True, stop=(s + 4 > 15))

            def emit_h2(s):
                """h2(s) = relu(bank(s+4)[100:110] + b2) -> fmov(s+6)
                rows 100:110, via DVE STT from base 96 (rows 96:100 are
                garbage lanes: for s<=9 overwritten by h1-ACT(s+6); for
                s>=10 they feed zero-weight stationary rows only)."""
                bk = banks[s + 4]
                nc.vector.scalar_tensor_tensor(
                    ptiles[(s + 6) // 2][96:110, s % 2, :],
                    bk[96:110, :], bq2_ap, wsc[96:110, :],
                    op0=mybir.AluOpType.add, op1=mybir.AluOpType.max)

            def emit_y_early(s):
                """y(s-6) for s=6..9: bank(s+4) rows 116:126 + b3 ->
                store. Runs in ACT's idle mid-loop windows."""
                bk = banks[s + 4]
                ot = opool.tile([126, TN], F32, tag="ot", name=f"ot{s-6}")
                nc.scalar.activation(ot[96:126, :], bk[96:126, :],
                                     ident, bias=bq_ap)
                nc.gpsimd.dma_start(
                    yt[:, (s - 6) * TN:(s - 5) * TN], ot[116:126, :]
                )
                ots[s - 6] = ot

            ybA = None
            for p in range(NPAIR):
                t0, t1 = 2 * p, 2 * p + 1
                alloc_ptile(p + 2)
                fm = ptiles[p]
                last = p == NPAIR - 1
                # fused passes for the pair-before-last: every dependency
                # (h1 of pair p-2, h2 writes from pair p-1) is at least a
                # full pair old, so the PE never waits here.
                if p >= 2:
                    emit_fused(2 * p - 4)
                    emit_fused(2 * p - 3)
                if last:
                    # L2-drain passes for s=12,13 (h1(12),h1(13) from
                    # pair 6; h2(6),h2(7) long done) + the first two
                    # stacked L3 passes (slots 12,13 fully written),
                    # + their h2 relus into fmov slots 18,19.
                    alloc_ptile(10)
                    emit_fused(12)
                    emit_fused(13)
                    ybA = psY.tile([126, TN], F32, tag="py", name="ybA")
                    mm(ybA[0:100, :], syA_ap(0), fmov(12)[0:110, :],
                       start=True, stop=False)
                    mm(ybA[0:100, :], syA_ap(1), fmov(13)[0:110, :],
                       start=False, stop=False)
                    emit_h2(12)
                    emit_h2(13)

                if p == 0:
                    xmA = xm0  # DMAs already issued up top, split
                    xmB = xpool.tile([128, NKC, TN], F8E3, tag="xm")
                    nc.sync.dma_start(xmB[:], xt_main[t1])
                    order = [(t0, xmA), (t1, xmB)]
                else:
                    xmA = xpool.tile([128, NKC, TN], F8E3, tag="xm")
                    xmB = xpool.tile([128, NKC, TN], F8E3, tag="xm")
                    if last:
                        # supertile 15 first so its h1/L2/h2 chain
                        # completes during supertile 14's chunks
                        nc.sync.dma_start(xmB[:], xt_main[t1])
                        nc.sync.dma_start(xmA[:], xt_main[t0])
                        order = [(t1, xmB), (t0, xmA)]
                    else:
                        nc.sync.dma_start(xmA[:], xt_main[t0])
                        nc.sync.dma_start(xmB[:], xt_main[t1])
                        order = [(t0, xmA), (t1, xmB)]

                for t, xm in order:
                    for k in range(NKC):
                        mm(banks[t][0:H1, :],
                           wb_s[:, k * H1:(k + 1) * H1],
                           xm[:, k, :], start=False, stop=(k == NKC - 1))
                    nc.scalar.activation(fm[0:H1, t % 2, :],
                                         banks[t][0:H1, :],
                                         relu, bias=b1_ap)
                    if last and t == t1:
                        # L2-drain(15) right behind h1(15): its h2 is
                        # then ready during supertile 14's chunks, and
                        # the y(15) stacked pass can open group B early
                        emit_fused(15)
                        emit_h2(15)

                # deferred bank readers for this pair's fused outputs
                if 2 <= p < 7:
                    s0, s1 = 2 * p - 4, 2 * p - 3
                    emit_h2(s0)
                    emit_h2(s1)
                    if s0 >= 6:
                        emit_y_early(s0)
                        emit_y_early(s1)
                if not last:
                    del banks[t0], banks[t1]

            # ---- drain ----
            # L2-drain(14); h2(10),h2(11) (their banks' rows 96:100 are
            # supertile-14/15 chunk accum, so they gate on those chunks);
            # h2(14); remaining stacked L3 passes; two wide ACTs; two
            # stores on separate queues.
            emit_fused(14)
            emit_h2(10)
            emit_h2(11)
            emit_h2(14)

            # stacked L3 passes j=2..7 -> ybA rows 10j..10j+10 (y(8..13))
            for j in range(2, 8):
                s = 6 + j
                mm(ybA[0:100, :], syA_ap(j), fmov(s + 6)[0:110, :],
                   start=False, stop=(j == 7))
            otA = opool.tile([126, TN], F32, tag="ot", name="otA")
            nc.scalar.activation(otA[0:80, :], ybA[0:80, :],
                                 ident, bias=b3rA_ap)
            nc.gpsimd.dma_start(yt2[:], otA[0:80, :])

            # group B: rows 0:10 y(4), 10:20 y(5), 20:30 y(14),
            # 30:40 y(15). h2(s) lives in fmov slot s+6; ptile 5 (slots
            # 10,11) survives pool cycling — only 11 ptiles exist for 6
            # slots. Pass order tracks data readiness: y(15)'s h2 is
            # ready first, y(14)'s last.
            ybB = psY.tile([126, TN], F32, tag="py", name="ybB")
            mm(ybB[0:40, :], syB_ap(3), fmov(21)[0:110, :],
               start=True, stop=False)
            mm(ybB[0:40, :], syB_ap(0), fmov(10)[0:110, :],
               start=False, stop=False)
            mm(ybB[0:40, :], syB_ap(1), fmov(11)[0:110, :],
               start=False, stop=False)
            mm(ybB[0:40, :], syB_ap(2), fmov(20)[0:110, :],
               start=False, stop=True)
            otB = opool.tile([126, TN], F32, tag="ot", name="otB")
            nc.scalar.activation(otB[0:40, :], ybB[0:40, :],
                                 ident, bias=b3rB_ap)
            nc.sync.dma_start(yt3[:], otB[0:40, :])

    nc.compile()
    return nc


def _fold_conv_into_w1(conv_w: np.ndarray, w1: np.ndarray) -> np.ndarray:
    """W1f[784,100] such that x @ W1f == conv(x).reshape(B,676) @ w1."""
    c = np.zeros((NF, 26 * 26), dtype=np.float64)
    for di in range(3):
        for dj in range(3):
            ii, jj = np.meshgrid(np.arange(26), np.arange(26), indexing="ij")
            src = (ii + di) * 28 + (jj + dj)
            dst = ii * 26 + jj
            c[src.ravel(), dst.ravel()] += np.float64(conv_w[di, dj])
    return (c @ w1.astype(np.float64)).astype(np.float32)


def _prep_in_maps(x, conv_w, w1, b1, w2, b2, w3, b3):
    x = np.asarray(x, dtype=np.float32)
    conv_w = np.asarray(conv_w, dtype=np.float32)
    w1 = np.asarray(w1, dtype=np.float32)
    b1 = np.asarray(b1, dtype=np.float32)
    w2 = np.asarray(w2, dtype=np.float32)
    b2 = np.asarray(b2, dtype=np.float32)
    w3 = np.asarray(w3, dtype=np.float32)
    b3 = np.asarray(b3, dtype=np.float32)

    w1f = _fold_conv_into_w1(conv_w, w1)  # [784, 100]
    # main chunks: feature f = k*128 + p -> [128, 600]
    w1m = np.ascontiguousarray(
        w1f[: 128 * NKC].reshape(NKC, 128, H1).transpose(1, 0, 2)
    ).astype(NP_BF16).reshape(128, NKC * H1)
    w1t = w1f[128 * NKC:].astype(NP_BF16)  # [16, 100]

    blob = np.zeros((128, WBW), np.uint16)
    blob[:, _C_W1M:_C_W1M + NKC * H1] = w1m.view(np.uint16)
    # fused stationary S[126,126]
    s_blk = np.zeros((126, 126), np.float32)
    s_blk[0:H1, 100:110] = w2
    s_blk[100:110, 116:126] = w3
    s_blk[110:126, 0:H1] = w1t.astype(np.float32)
    blob[0:126, _C_FS:_C_FS + 126] = s_blk.astype(NP_BF16).view(np.uint16)
    blob[0:KT, _C_W1T:_C_W1T + H1] = w1t.view(np.uint16)
    bias_rows = np.zeros((126, 1), np.float32)
    bias_rows[0:H1, 0] = b1
    bias_rows[100:110, 0] = b2
    bias_rows[116:126, 0] = b3
    blob[0:126, _C_B:_C_B + 2] = bias_rows.view(np.uint16)
    # stacked-L3 stationary SYW: w3 at rows 100:110, cols 90:100
    syw = np.zeros((110, 190), np.float32)
    syw[100:110, 90:100] = w3
    blob[0:110, _C_SY:_C_SY + 190] = syw.astype(NP_BF16).view(np.uint16)
    b3rep = np.zeros((100, 1), np.float32)
    b3rep[:, 0] = np.tile(b3, 10)
    blob[0:100, _C_B3R:_C_B3R + 2] = b3rep.view(np.uint16)
    shared = {"wblob": blob.view(NP_BF16)}

    xb = x.astype(NP_F8E3)  # cast once, full batch
    in_maps = []
    for core in range(N_CORES):
        xc = xb[core * BC:(core + 1) * BC]  # [8192, 784] f8e3
        xct = xc.reshape(NT, TN, NF).transpose(0, 2, 1)  # [NT, NF, TN]
        xt_main = np.ascontiguousarray(
            xct[:, : 128 * NKC].reshape(NT, NKC, 128, TN).transpose(0, 2, 1, 3)
        )  # [NT, 128, NKC, TN]
        tails = xct[:, 128 * NKC:].astype(NP_BF16)  # [NT, KT, TN]
        # fmov slot s holds xtail(s+4); slots 12..15 stay zero
        xt_tail = np.zeros((KT, 16, TN), NP_BF16)
        xt_tail[:, 0:12, :] = tails[4:16].transpose(1, 0, 2)
        xt_tl03 = np.ascontiguousarray(tails[0:4].transpose(1, 0, 2))
        in_maps.append({"xt_main": xt_main, "xt_tail": xt_tail,
                        "xt_tl03": xt_tl03, **shared})
    return in_maps


_NC = None


def _get_nc():
    global _NC
    if _NC is None:
        _NC = _build_nc()
    return _NC


def _assemble(results):
    out = np.empty((B, HO), dtype=np.float32)
    for i in range(N_CORES):
        o = out[i * BC:(i + 1) * BC]
        r = results[i]
        # y(0..3) from yt, y(4,5,14,15) from yt3, y(6..13) from yt2
        o[0:4 * TN] = r["yt"].T
        y3 = r["yt3"].reshape(4, HO, TN)
        o[4 * TN:5 * TN] = y3[0].T
        o[5 * TN:6 * TN] = y3[1].T
        o[6 * TN:14 * TN] = r["yt2"].reshape(8, HO, TN).transpose(
            0, 2, 1).reshape(8 * TN, HO)
        o[14 * TN:15 * TN] = y3[2].T
        o[15 * TN:16 * TN] = y3[3].T
    return out


def kernel(x, conv_w, w1, b1, w2, b2, w3, b3):
    in_maps = _prep_in_maps(x, conv_w, w1, b1, w2, b2, w3, b3)
    nc = _get_nc()
    res = run_bass_kernel_spmd(nc, in_maps, core_ids=list(range(N_CORES)))
    return _assemble(res.results)


if __name__ == "__main__":
    rng = np.random.default_rng(0)
    inputs = {
        "x": rng.standard_normal((B, NF), dtype=np.float32),
        "conv_w": np.ones((3, 3), dtype=np.float32),
        "w1": (rng.standard_normal((676, H1)) * 0.04).astype(np.float32),
        "b1": np.zeros(H1, dtype=np.float32),
        "w2": (rng.standard_normal((H1, HO)) * 0.1).astype(np.float32),
        "b2": np.zeros(HO, dtype=np.float32),
        "w3": (rng.standard_normal((HO, HO)) * 0.3).astype(np.float32),
        "b3": np.zeros(HO, dtype=np.float32),
    }
    out = kernel(**inputs)
    print(out.shape, out.dtype)
